# revision 22
# baseline (speedup 1.0000x reference)
"""GCN conv (linear -> weighted gather -> segment-sum by dst) on 8 trn2 cores.

Math: out = segment_sum((x @ W.T + b)[src] * w[:, None], dst, N)

Strategy per core (nodes range-partitioned across cores, edges by dst):
  - Host bin-packs each core's 12500 nodes into 98 blocks of 128 slots,
    balancing each block's per-src-window edge counts so a uniform
    (SPMD-safe) chunk schedule works; the node->slot permutation is undone
    on the host after the run.
  - Device gathers x rows by src with bulk `dma_gather` calls (int16
    indices force 4 source windows of <=32768 rows; rows padded to 256 B
    with the ones column baked in). One call per (block-group, window)
    replaces the old per-chunk indirect DMAs whose ~1us/instruction SWDGE
    fixed cost dominated.
  - Weighted one-hot matmul segment-sums pre-linear features per block:
        pst[feat, slot] += sum_p gx[p, feat] * (w_p * [rel_p == slot])
    giving S_ext = [segsum(w*x) | segsum(w)] per block, pre-transposed.
  - A second matmul applies the linear layer: out = S_ext @ [W | b]^T.
  All matmul operands are bf16 (fp32 matmul runs at 1/4 rate).
"""

import bass_rust
import ml_dtypes
import numpy as np

from concourse import bass, library_config, mybir, tile
from concourse.bass_utils import run_bass_kernel_spmd

P = 128
NCORES = 8
N, E, D = 100000, 1200000, 64
NODES_PER_CORE = N // NCORES  # 12500
NB = (NODES_PER_CORE + P - 1) // P  # 98 blocks of 128 node slots
NPAD = NB * P  # 12544
GROUP = 7  # blocks per gather group (NB % GROUP == 0)
NGROUP = NB // GROUP  # 14
WIN = 32768  # src window size (int16 index limit)
NW = 4  # number of src windows: 3*32768 + 1696
ROW = 128  # padded x row length in bf16 elems (256 B, dma_gather granule)

BF16 = ml_dtypes.bfloat16

_wait_counter = [0]


def _split_multi_waits(nc):
    """Installed walrus rejects >1 sync wait per instruction; park excess
    waits on fresh single-wait NoOps inserted before the owner (same
    engine, so in-order execution preserves semantics)."""
    for fn in nc.m.functions:
        for bb in fn.blocks:
            insts = bb.instructions
            if not any(
                i.sync_info is not None and len(i.sync_info.on_wait) > 1
                for i in insts
            ):
                continue
            out = []
            for inst in insts:
                si = inst.sync_info
                waits = list(si.on_wait) if si is not None else []
                if len(waits) > 1:
                    for wv in waits[:-1]:
                        _wait_counter[0] += 1
                        nop = mybir.InstNoOp(
                            name=f"waitsplit-{_wait_counter[0]}",
                            engine=inst.engine,
                        )
                        nop.sync_info = bass_rust.SyncInfo(
                            on_wait=[wv], on_update=[]
                        )
                        out.append(nop)
                    inst.sync_info = bass_rust.SyncInfo(
                        on_wait=[waits[-1]], on_update=list(si.on_update)
                    )
                out.append(inst)
            bb.instructions = out


class _TC(tile.TileContext):
    def __exit__(self, *args):
        ret = super().__exit__(*args)
        _split_multi_waits(self.nc)
        return ret


def _build_program(Bw: tuple[int, ...]):
    B = sum(Bw)  # chunks per block
    C = NB * B  # chunks per core
    IC = C * P // 16  # idx table columns (16-wrapped)
    woff = [GROUP * sum(Bw[:w]) for w in range(NW)]  # chunk col offset of
    # window-w section within a group
    f32 = mybir.dt.float32
    bf16 = mybir.dt.bfloat16
    nc = bass.Bass(num_swdge_queues=4)
    x_p = nc.declare_dram_parameter("x", [N, ROW], bf16, isOutput=False)
    idxT_p = nc.declare_dram_parameter("idxT", [P, IC], mybir.dt.int16, isOutput=False)
    relT_p = nc.declare_dram_parameter("relT", [P, C], f32, isOutput=False)
    wT_p = nc.declare_dram_parameter("wT", [P, C], f32, isOutput=False)
    relNegT_p = nc.declare_dram_parameter("relNegT", [P, C], f32, isOutput=False)
    wNegT_p = nc.declare_dram_parameter("wNegT", [P, C], f32, isOutput=False)
    wext_p = nc.declare_dram_parameter("wext", [D + 1, D], bf16, isOutput=False)
    iota_p = nc.declare_dram_parameter("iota", [P, P], bf16, isOutput=False)
    out_p = nc.declare_dram_parameter("out", [NPAD, D], f32, isOutput=True)

    with _TC(nc) as tc:
        with (
            tc.tile_pool(name="const", bufs=1) as cpool,
            tc.tile_pool(name="gx0", bufs=2) as gxpool0,
            tc.tile_pool(name="gx1", bufs=2) as gxpool1,
            tc.tile_pool(name="gx2", bufs=2) as gxpool2,
            tc.tile_pool(name="gx3", bufs=2) as gxpool3,
            tc.tile_pool(name="oh", bufs=6) as ohpool,
            tc.tile_pool(name="sq", bufs=4) as sqpool,
            tc.tile_pool(name="stsb", bufs=3) as stpool,
            tc.tile_pool(name="outsb", bufs=3) as opool,
            tc.tile_pool(name="pst", bufs=3, space="PSUM") as pstpool,
            tc.tile_pool(name="pout", bufs=2, space="PSUM") as poutpool,
        ):
            gxpools = [gxpool0, gxpool1, gxpool2, gxpool3]
            # dma_gather's Q7 ucode lives in the mlp library
            nc.gpsimd.load_library(library_config.mlp)
            iota_sb = cpool.tile([P, P], bf16)
            nc.sync.dma_start(out=iota_sb[:], in_=iota_p[:])
            wext_sb = cpool.tile([D + 1, D], bf16)
            nc.sync.dma_start(out=wext_sb[:], in_=wext_p[:])
            # one-shot [128, C] loads of this size crash neuronxcc's
            # DataLocalityOpt; slice them into <=98-column pieces
            idxT_sb = cpool.tile([P, IC], mybir.dt.int16)
            relT_sb = cpool.tile([P, C], f32)
            wT_sb = cpool.tile([P, C], f32)
            relNegT_sb = cpool.tile([P, C], f32)
            wNegT_sb = cpool.tile([P, C], f32)
            for s in range(0, C, 98):
                e = min(C, s + 98)
                nc.sync.dma_start(out=relT_sb[:, s:e], in_=relT_p[:, s:e])
                nc.sync.dma_start(out=wT_sb[:, s:e], in_=wT_p[:, s:e])
                nc.sync.dma_start(out=relNegT_sb[:, s:e], in_=relNegT_p[:, s:e])
                nc.sync.dma_start(out=wNegT_sb[:, s:e], in_=wNegT_p[:, s:e])
            for s in range(0, IC, 384):
                e = min(IC, s + 384)
                nc.sync.dma_start(out=idxT_sb[:, s:e], in_=idxT_p[:, s:e])

            # gathers are split into <=1024-index sub-calls (the dma_gather
            # ucode wedges the device somewhere between 1024 and 1792 idxs
            # per call). One register per distinct num_idxs value (a fresh
            # to_reg per call exhausts the Pool register pool).
            sub_sizes = set()
            for bw in Bw:
                n_tot = GROUP * bw * P
                while n_tot > 0:
                    sub_sizes.add(min(1024, n_tot))
                    n_tot -= min(1024, n_tot)
            nreg = {n: nc.gpsimd.to_reg(n) for n in sorted(sub_sizes)}

            qload = [0.0, 0.0, 0.0, 0.0]
            for g in range(NGROUP):
                gx = []
                for w in range(NW):
                    n_tot = GROUP * Bw[w] * P  # idxs in this window section
                    t = gxpools[w].tile([P, GROUP * Bw[w], ROW], bf16)
                    sec = (g * GROUP * B * P + woff[w] * P) // 16  # idxT col start
                    w0 = w * WIN
                    w1 = min(N, (w + 1) * WIN)
                    off = 0
                    while off < n_tot:
                        n_sub = min(1024, n_tot - off)
                        qq = min(range(4), key=lambda q: qload[q])
                        qload[qq] += n_sub
                        nc.gpsimd.dma_gather(
                            t[:, off // P : (off + n_sub) // P, :],
                            x_p[w0:w1, :],
                            idxT_sb[:, sec + off // 16 : sec + (off + n_sub) // 16],
                            n_sub,
                            nreg[n_sub],
                            ROW,
                            queue_num=qq,
                        )
                        off += n_sub
                    gx.append(t)

                for bl in range(GROUP):
                    blk = g * GROUP + bl
                    pst = pstpool.tile([D + 1, P], f32)
                    for w in range(NW):
                        for j in range(Bw[w]):
                            cc = g * GROUP * B + woff[w] + bl * Bw[w] + j
                            oh = ohpool.tile([P, P], bf16)
                            # oh[p, n] = w[p] * (rel_dst[p] == n); ~1/3 of
                            # chunks build it on the idle ACT engine as
                            # relu(w - w*(iota - rel)^2), rest on DVE
                            if cc % 3 == 2:
                                sq = sqpool.tile([P, P], bf16)
                                nc.scalar.activation(
                                    sq[:],
                                    iota_sb[:],
                                    mybir.ActivationFunctionType.Square,
                                    relNegT_sb[:, cc : cc + 1],
                                    1.0,
                                )
                                nc.scalar.activation(
                                    oh[:],
                                    sq[:],
                                    mybir.ActivationFunctionType.Relu,
                                    wT_sb[:, cc : cc + 1],
                                    wNegT_sb[:, cc : cc + 1],
                                )
                            else:
                                nc.vector.tensor_scalar(
                                    out=oh[:],
                                    in0=iota_sb[:],
                                    scalar1=relT_sb[:, cc : cc + 1],
                                    scalar2=wT_sb[:, cc : cc + 1],
                                    op0=mybir.AluOpType.is_equal,
                                    op1=mybir.AluOpType.mult,
                                )
                            # pst[feat, n] += sum_p gx[p, col, feat] * oh[p, n]
                            nc.tensor.matmul(
                                pst[:],
                                lhsT=gx[w][:, bl * Bw[w] + j, 0 : D + 1],
                                rhs=oh[:],
                                start=(w == 0 and j == 0),
                                stop=(w == NW - 1 and j == Bw[NW - 1] - 1),
                            )
                    st_sb = stpool.tile([D + 1, P], bf16)
                    nc.scalar.copy(out=st_sb[:], in_=pst[:])
                    pout = poutpool.tile([P, D], f32)
                    # out[n, dout] = sum_k st[k, n] * wext[k, dout]
                    nc.tensor.matmul(
                        pout[:], lhsT=st_sb[:], rhs=wext_sb[:], start=True, stop=True
                    )
                    out_sb = opool.tile([P, D], f32)
                    nc.scalar.copy(out=out_sb[:], in_=pout[:])
                    nc.sync.dma_start(
                        out=out_p[blk * P : (blk + 1) * P, :], in_=out_sb[:]
                    )
    # materialize ISA bytes for the pseudo library-reload (walrus codegen
    # rejects InstISA with empty instr otherwise)
    mybir.codegen_inst_isa_subclasses(nc)
    return nc


def _balance_blocks(degw: np.ndarray, caps: np.ndarray) -> np.ndarray:
    """Assign each local node a slot in [0, NPAD) such that every block's
    per-window edge counts stay near-balanced (<= caps elementwise if
    possible). degw: [n_nodes, NW] per-window in-degree. Returns perm with
    perm[node_local] = slot. Greedy: heaviest nodes first, into the block
    minimizing the worst fractional window load."""
    n = degw.shape[0]
    order = np.argsort(-degw.sum(axis=1), kind="stable")
    load = np.zeros((NB, degw.shape[1]), dtype=np.float64)
    used = np.zeros(NB, dtype=np.int64)
    perm = np.empty(n, dtype=np.int64)
    capf = caps.astype(np.float64)
    for node in order:
        cost = ((load + degw[node]) / capf).max(axis=1)
        cost[used >= P] = np.inf
        b = int(np.argmin(cost))
        perm[node] = b * P + used[b]
        used[b] += 1
        load[b] += degw[node]
    return perm


def kernel(x, src, dst, w, W, b):
    x = np.asarray(x, dtype=np.float32)
    src = np.asarray(src).astype(np.int64)
    dst = np.asarray(dst).astype(np.int64)
    w = np.asarray(w, dtype=np.float32)
    W = np.asarray(W, dtype=np.float32)
    b = np.asarray(b, dtype=np.float32)

    x_pad = np.zeros((N, ROW), np.float32)
    x_pad[:, :D] = x
    x_pad[:, D] = 1.0
    x_bf = x_pad.astype(BF16)

    win_of = src >> 15  # windows of 32768; 100000 < 4*32768
    core_of = dst // NODES_PER_CORE
    per_core = []
    maxcnt = np.zeros(NW, dtype=np.int64)
    caps = np.array([512, 512, 512, 128], dtype=np.int64)
    for c in range(NCORES):
        m = core_of == c
        s_c = src[m]
        d_c = dst[m] - c * NODES_PER_CORE
        w_c = w[m]
        wn_c = win_of[m]
        degw = np.zeros((NODES_PER_CORE, NW), dtype=np.int64)
        np.add.at(degw, (d_c, wn_c), 1)
        perm = _balance_blocks(degw, caps)  # node_local -> slot
        slot = perm[d_c]
        blk = slot >> 7
        # sort edges by (block, window)
        key = blk * NW + wn_c
        order = np.argsort(key, kind="stable")
        s_c, w_c, slot, blk, wn_s = (
            s_c[order],
            w_c[order],
            slot[order],
            blk[order],
            wn_c[order],
        )
        key = key[order]
        counts = np.bincount(key, minlength=NB * NW).reshape(NB, NW)
        per_core.append((s_c, w_c, slot, key, counts, perm))
        maxcnt = np.maximum(maxcnt, counts.max(axis=0))
    Bw = tuple(int(-(-int(mc) // P)) for mc in maxcnt)
    B = sum(Bw)
    C = NB * B
    IC = C * P // 16
    woff = [GROUP * sum(Bw[:ww]) for ww in range(NW)]
    bw_off = np.concatenate([[0], np.cumsum(np.array(Bw))])[:NW]  # chunk
    # offset of window w inside one block's chunk list (b-local, see below)

    wext = np.ascontiguousarray(np.concatenate([W, b[:, None]], axis=1).T).astype(BF16)
    iota = np.ascontiguousarray(
        np.tile(np.arange(P, dtype=np.float32), (P, 1))
    ).astype(BF16)

    in_maps = []
    for c in range(NCORES):
        s_c, w_c, slot, key, counts, perm = per_core[c]
        ccounts = counts.reshape(-1)  # [NB*NW] in key order
        run_start = np.zeros(NB * NW, dtype=np.int64)
        run_start[1:] = np.cumsum(ccounts)[:-1]
        within = np.arange(len(slot), dtype=np.int64) - run_start[key]
        blk = key // NW
        wn = key % NW
        g = blk // GROUP
        bl = blk % GROUP
        # chunk column of this edge
        cc = g * GROUP * B + np.array(woff)[wn] + bl * np.array(Bw)[wn] + within // P
        pp = within % P
        flat_rel = np.zeros(C * P, dtype=np.float32)
        flat_w = np.zeros(C * P, dtype=np.float32)
        flat_rel[cc * P + pp] = (slot & 127).astype(np.float32)
        flat_w[cc * P + pp] = w_c
        # idx table: call (g, w) local slot i = (bl*Bw[w] + within//P)*P + pp
        # wrapped at 16: partition i%16 (replicated x8), col sec + i//16
        i_local = (bl * np.array(Bw)[wn] + within // P) * P + pp
        sec = (g * GROUP * B * P + np.array(woff)[wn] * P) // 16
        idx_cols = sec + i_local // 16
        idx_rows = i_local % 16
        idxT16 = np.zeros((16, IC), dtype=np.int16)
        idxT16[idx_rows, idx_cols] = (s_c - (wn << 15)).astype(np.int16)
        idxT = np.ascontiguousarray(np.tile(idxT16, (8, 1)))
        in_maps.append(
            {
                "x": x_bf,
                "idxT": idxT,
                "relT": np.ascontiguousarray(flat_rel.reshape(C, P).T),
                "wT": np.ascontiguousarray(flat_w.reshape(C, P).T),
                "relNegT": np.ascontiguousarray(-flat_rel.reshape(C, P).T),
                "wNegT": np.ascontiguousarray(-flat_w.reshape(C, P).T),
                "wext": wext,
                "iota": iota,
            }
        )

    nc = _build_program(Bw)
    global _last_nc, _last_in_maps
    _last_nc, _last_in_maps = nc, in_maps
    results = run_bass_kernel_spmd(nc, in_maps, list(range(NCORES))).results
    out = np.concatenate(
        [
            results[c]["out"][per_core[c][5][:NODES_PER_CORE]]
            for c in range(NCORES)
        ],
        axis=0,
    )
    return out.astype(np.float32)
